# revision 24
# baseline (speedup 1.0000x reference)
import sys
sys.path.insert(0, '/opt/trn_rl_repo')
import numpy as np
import concourse.bass as bass
import concourse.bacc as bacc
import concourse.tile as tile
from concourse import mybir
from concourse.bass_utils import run_bass_kernel_spmd

F32 = mybir.dt.float32
F16 = mybir.dt.float16
AF = mybir.ActivationFunctionType
ALU = mybir.AluOpType

U = 400       # LSTM units
KATT = 10     # attention gaussians
NCHARS = 73   # alphabet
NMIX = 20     # GMM components
UC = 50       # char positions
NB = 4        # batch per core
NCORES = 8
NOUT = 6 * NMIX + 1  # 121

# m-tiles: per gate [128,128,128,16] -> 16 m-tiles, psum z [128, 64]
MW = [128, 128, 128, 16] * 4
SOFF = [400 * (m // 4) + 128 * (m % 4) for m in range(16)]   # W col offsets (unpadded)
XWROWS = 99  # xw moving tile: w @0:73, junk, x @96:99 (x dst partition must be %32)

_CACHE = {}


def _pack_wblocks(tiles):
    """list of [rows<=128, 1600] -> [128, ntiles*1600] k-major."""
    out = np.zeros((128, len(tiles) * 1600), np.float32)
    for k, tl in enumerate(tiles):
        out[0 : tl.shape[0], k * 1600 : k * 1600 + 1600] = tl
    return out


def _build_program(T):
    nc = bacc.Bacc("TRN2", target_bir_lowering=False, debug=False, num_devices=NCORES)

    dW1 = nc.dram_tensor("W1", [128, 5 * 1600], F16, kind="ExternalInput").ap()
    dW2 = nc.dram_tensor("W2", [128, 9 * 1600], F16, kind="ExternalInput").ap()
    dW3 = nc.dram_tensor("W3", [128, 9 * 1600], F16, kind="ExternalInput").ap()
    dPB = nc.dram_tensor("PB", [128, 9 * 16], F32, kind="ExternalInput").ap()
    dXT = nc.dram_tensor("XT", [3, (T + 1) * NB], F32, kind="ExternalInput").ap()
    dWATT = nc.dram_tensor("WATT", [128, 4 * 30], F32, kind="ExternalInput").ap()
    dV3 = nc.dram_tensor("V3", [1, 3 * UC], F32, kind="ExternalInput").ap()
    dOHB = nc.dram_tensor("OHB", [UC, NB * NCHARS], F32, kind="ExternalInput").ap()
    dWMDN = nc.dram_tensor("WMDN", [128, 4 * 200], F32, kind="ExternalInput").ap()
    dHB = nc.dram_tensor("HB", [128, 16], F16, kind="ExternalInput").ap()
    dHBR = nc.dram_tensor("HBR", [128, 512], F32, kind="ExternalInput").ap()
    dOUT1 = nc.dram_tensor("OUT1", [128, T * NB], F32, kind="ExternalOutput").ap()
    dOUT2 = nc.dram_tensor("OUT2", [72, T * NB], F32, kind="ExternalOutput").ap()

    with tile.TileContext(nc) as tc:
        with tc.tile_pool(name="statics", bufs=1) as statics, \
             tc.tile_pool(name="states", bufs=1) as states:

            sW1 = statics.tile([128, 5 * 1600], F16)
            sW2 = statics.tile([128, 9 * 1600], F16)
            sW3 = statics.tile([128, 9 * 1600], F16)
            sPB = statics.tile([128, 9 * 16], F32)
            sXT = statics.tile([3, (T + 1) * NB], F32)
            sWATT = statics.tile([128, 4 * 30], F32)
            sV3 = statics.tile([1, 3 * UC], F32)
            sOHB = statics.tile([UC, NB * NCHARS], F32)
            sWMDN = statics.tile([128, 4 * 200], F32)
            sHB = statics.tile([128, 16], F16)

            # h3all has T+1 slots: slot s holds h3 of step s-1 (cell2 deferred)
            h3all = states.tile([128, (T + 1) * 16], F32)
            OUTS1 = states.tile([128, T * NB], F32)
            OUTS2 = states.tile([72, T * NB], F32)
            h = [states.tile([128, 16], F16, name=f"h{i}") for i in range(3)]
            # h1 replica, batch at col {128k + 32b}: att matmul stationary
            h1r = states.tile([128, 512], F32, name="h1r")
            c = [states.tile([128, 16], F32, name=f"c{i}") for i in range(3)]
            xw = [states.tile([XWROWS, NB], F16, name=f"xw{i}") for i in range(3)]
            kap = states.tile([1, 4 * KATT], F32)  # (b, k)

            for dst, src in [(sW1, dW1), (sW2, dW2), (sW3, dW3), (sPB, dPB),
                             (sXT, dXT), (sWATT, dWATT), (sV3, dV3),
                             (sOHB, dOHB), (sWMDN, dWMDN), (sHB, dHB),
                             (h1r, dHBR)]:
                nc.gpsimd.dma_start(out=dst[:], in_=src[:])

            for tl in c + xw:
                nc.vector.memset(tl[:], 0.0)
            nc.vector.memset(kap[:], 0.0)
            nc.vector.memset(h3all[:], 0.0)
            # h init: zeros except bias constant 1.0 at (p96, blk3 cols)
            for tl in h:
                nc.vector.tensor_copy(out=tl[:], in_=sHB[:])
            nc.vector.memset(h3all[96:128, :], 1.0)

            with tc.tile_pool(name="psum", bufs=1, space="PSUM") as psum, \
                 tc.tile_pool(name="scratch", bufs=2) as scratch:

                # full-bank z tiles (512 f32 = 2KB) so each owns its zero region
                z = [psum.tile([128, 512], F32, name=f"z{i}") for i in range(3)]
                attp = psum.tile([97, 30], F32)   # att out, batch b at row 32b
                argp = psum.tile([UC, NB * KATT], F32)
                wp = psum.tile([NCHARS, NB], F32)
                for tl in z:
                    nc.vector.memset(tl[:], 0.0)

                zS = {1: None, 2: None}
                _sc = {}
                for _t in ["pc0", "zi_", "ti", "ig", "pc1", "zf_", "tf", "fg",
                           "tg", "m1", "m2", "pc2", "zo_", "to", "og", "tcn",
                           "zi2", "zf2", "zg2", "zo2", "zgc"]:
                    _sc[_t] = scratch.tile([128, 16], F32, tag=_t, name=_t)
                PC = {1: None, 2: None}
                for _i in (1, 2):
                    zS[_i] = scratch.tile([128, 64], F32, tag=f"zS{_i}",
                                          name=f"zS{_i}")
                    nc.vector.memset(zS[_i][:], 0.0)
                    PC[_i] = scratch.tile([128, 64], F32, tag=f"PC{_i}",
                                          name=f"PC{_i}")
                    nc.vector.memset(PC[_i][:], 0.0)
                for _t, _shp in [("attR", [1, NB * 30]), ("E", [1, NB * 30]),
                                 ("kap2", [1, NB * KATT]),
                                 ("bk", [1, NB * KATT]), ("A_", [1, NB * KATT]),
                                 ("bk2", [1, NB * KATT]), ("B_", [1, NB * KATT]),
                                 ("C_", [1, NB * KATT]), ("P", [UC, NB * KATT]),
                                 ("phi", [UC, NB])]:
                    _sc[_t] = scratch.tile(_shp, F32, tag=_t, name=_t)

                def st(tag):
                    return _sc[tag]

                def layer_mms(l, sW, movs, cb=0):
                    # per-m-tile accumulation groups (start/stop per m): the
                    # group check serializes groups, preserving long
                    # same-psum-address runs (HW chains those at ~27ns;
                    # address switches cost ~95ns pipeline drain). cb: psum
                    # column base — L2/L3 self and rest passes write separate
                    # halves of the z bank, summed by the (split) cell.
                    for m in range(16):
                        w0, mw = int(SOFF[m]), MW[m]
                        for ki, (mov, kidx) in enumerate(movs):
                            kp = mov.shape[0]
                            nc.tensor.matmul(
                                z[l][0:mw, cb + 4 * m : cb + 4 * m + 4],
                                sW[0:kp, kidx * 1600 + w0 : kidx * 1600 + w0 + mw],
                                mov,
                                start=(ki == 0),
                                stop=(ki == len(movs) - 1),
                            )

                def mv_self(hh):
                    return [(hh[:, 0:4], 0), (hh[:, 4:8], 1),
                            (hh[:, 8:12], 2), (hh[:, 12:16], 3)]

                def mv_skip(hh, base):
                    return [(hh[:, 0:4], base), (hh[:, 4:8], base + 1),
                            (hh[:, 8:12], base + 2), (hh[:, 12:16], base + 3)]

                def cell(l, slot16=None, split=False):
                    zt = z[l]
                    zRi, zRf = zt[:, 64:80], zt[:, 80:96]
                    zRg, zRo = zt[:, 96:112], zt[:, 112:128]
                    p2 = sPB[:, (3 * l + 2) * 16 : (3 * l + 2) * 16 + 16]
                    ct = c[l]
                    if split:
                        # zS staged to SBUF earlier (during attention): one
                        # on-chain add per gate, one PSUM operand each
                        zs = zS[l]
                        zi_ = st("zi_")
                        nc.vector.tensor_tensor(out=zi_[:], in0=zs[:, 0:16],
                                                in1=zRi, op=ALU.add)
                        zf_ = st("zf_")
                        nc.vector.tensor_tensor(out=zf_[:], in0=zs[:, 16:32],
                                                in1=zRf, op=ALU.add)
                        zg_ = st("zg2")
                        nc.vector.tensor_tensor(out=zg_[:], in0=zs[:, 32:48],
                                                in1=zRg, op=ALU.add)
                        # o-gate partial (peephole p2*c_new added later)
                        zo12 = st("zo2")
                        nc.vector.tensor_tensor(out=zo12[:], in0=zs[:, 48:64],
                                                in1=zRo, op=ALU.add)
                        zi_, zf_, zg_, zo_base = zi_[:], zf_[:], zg_[:], zo12[:]
                    else:
                        p0 = sPB[:, (3 * l + 0) * 16 : (3 * l + 0) * 16 + 16]
                        p1 = sPB[:, (3 * l + 1) * 16 : (3 * l + 1) * 16 + 16]
                        pc0 = st("pc0")
                        nc.vector.tensor_tensor(out=pc0[:], in0=p0, in1=ct[:],
                                                op=ALU.mult)
                        pc1 = st("pc1")
                        nc.vector.tensor_tensor(out=pc1[:], in0=p1, in1=ct[:],
                                                op=ALU.mult)
                        zi_ = st("zi_")
                        nc.vector.tensor_tensor(out=zi_[:], in0=pc0[:],
                                                in1=zt[:, 0:16], op=ALU.add)
                        zf_ = st("zf_")
                        nc.vector.tensor_tensor(out=zf_[:], in0=pc1[:],
                                                in1=zt[:, 16:32], op=ALU.add)
                        zi_, zf_ = zi_[:], zf_[:]
                        zg_, zo_base = zt[:, 32:48], zt[:, 48:64]
                    ti = st("ti")
                    nc.scalar.activation(out=ti[:], in_=zi_, func=AF.Tanh, scale=0.5)
                    tf = st("tf")
                    nc.scalar.activation(out=tf[:], in_=zf_, func=AF.Tanh, scale=0.5)
                    tg = st("tg")
                    nc.scalar.activation(out=tg[:], in_=zg_, func=AF.Tanh)
                    ig = st("ig")
                    nc.vector.tensor_scalar(out=ig[:], in0=ti[:], scalar1=0.5,
                                            scalar2=0.5, op0=ALU.mult, op1=ALU.add)
                    fg = st("fg")
                    nc.vector.tensor_scalar(out=fg[:], in0=tf[:], scalar1=0.5,
                                            scalar2=0.5, op0=ALU.mult, op1=ALU.add)
                    m1 = st("m1")
                    nc.vector.tensor_tensor(out=m1[:], in0=ig[:], in1=tg[:], op=ALU.mult)
                    m2 = st("m2")
                    nc.vector.tensor_tensor(out=m2[:], in0=fg[:], in1=ct[:], op=ALU.mult)
                    nc.vector.tensor_tensor(out=ct[:], in0=m1[:], in1=m2[:], op=ALU.add)
                    tcn = st("tcn")
                    nc.scalar.activation(out=tcn[:], in_=ct[:], func=AF.Tanh)
                    pc2 = st("pc2")
                    nc.vector.tensor_tensor(out=pc2[:], in0=p2, in1=ct[:], op=ALU.mult)
                    zo_ = st("zo_")
                    nc.vector.tensor_tensor(out=zo_[:], in0=pc2[:], in1=zo_base,
                                            op=ALU.add)
                    to = st("to")
                    nc.scalar.activation(out=to[:], in_=zo_[:], func=AF.Tanh, scale=0.5)
                    og = st("og")
                    nc.vector.tensor_scalar(out=og[:], in0=to[:], scalar1=0.5,
                                            scalar2=0.5, op0=ALU.mult, op1=ALU.add)
                    if l == 0:
                        # fp32 replica first: it gates the attention matmuls
                        hrb = h1r[:, 0:384].rearrange(
                            "p (k b s) -> p k b s", k=3, b=4)[:, :, :, 0:1]
                        ogb = og[:, 0:12].rearrange(
                            "p (k b s) -> p k b s", k=3, b=4)
                        tcb = tcn[:, 0:12].rearrange(
                            "p (k b s) -> p k b s", k=3, b=4)
                        nc.vector.tensor_tensor(out=hrb, in0=ogb, in1=tcb,
                                                op=ALU.mult)
                        hrr = h1r[0:16, 384:512].rearrange(
                            "p (b s) -> p b s", b=4)[:, :, 0:1]
                        ogr = og[0:16, 12:16].rearrange("p (b s) -> p b s", b=4)
                        tcr = tcn[0:16, 12:16].rearrange("p (b s) -> p b s", b=4)
                        nc.vector.tensor_tensor(out=hrr, in0=ogr, in1=tcr,
                                                op=ALU.mult)
                    # h = og*tcn; blk3 valid at partitions 0:16 (bias at p96)
                    nc.vector.tensor_tensor(out=h[l][:, 0:12], in0=og[:, 0:12],
                                            in1=tcn[:, 0:12], op=ALU.mult)
                    nc.vector.tensor_tensor(out=h[l][0:16, 12:16], in0=og[0:16, 12:16],
                                            in1=tcn[0:16, 12:16], op=ALU.mult)
                    if l == 2:
                        # fp32 h3 straight into its h3all slot
                        nc.vector.tensor_tensor(
                            out=h3all[:, bass.DynSlice(slot16, 12)],
                            in0=og[:, 0:12], in1=tcn[:, 0:12], op=ALU.mult)
                        nc.vector.tensor_tensor(
                            out=h3all[0:16, bass.DynSlice(slot16 + 12, 4)],
                            in0=og[0:16, 12:16], in1=tcn[0:16, 12:16], op=ALU.mult)

                def attention_mms():
                    # attp[32b + junk, 30] = h1r_k^T @ WATT_k (fp32, 4 fat mms)
                    for k in range(4):
                        stat = (h1r[0:128, 128 * k : 128 * k + 97] if k < 3
                                else h1r[0:128, 384:481])
                        kp = stat.shape[0]
                        nc.tensor.matmul(
                            attp[0:97, 0:30],
                            stat,
                            sWATT[0:kp, 30 * k : 30 * k + 30],
                            start=(k == 0), stop=(k == 3),
                        )

                def attention_tail(t):
                    attR = _sc["attR"]
                    for b in range(NB):
                        nc.vector.tensor_copy(
                            out=attR[0:1, 30 * b : 30 * b + 30],
                            in_=attp[32 * b : 32 * b + 1, 0:30])
                    E = _sc["E"]
                    nc.scalar.activation(out=E[:], in_=attR[:], func=AF.Exp)
                    Ev = E[0:1, :].rearrange("p (b x) -> p b x", b=NB)
                    av = attR[0:1, :].rearrange("p (b x) -> p b x", b=NB)
                    kapv = kap[0:1, :].rearrange("p (b k) -> p b k", b=NB)
                    kap2 = _sc["kap2"]
                    k2v = kap2[0:1, :].rearrange("p (b k) -> p b k", b=NB)
                    nc.vector.tensor_tensor(out=k2v, in0=kapv, in1=Ev[:, :, 20:30],
                                            op=ALU.add)
                    nc.vector.tensor_copy(out=kap[:], in_=kap2[:])
                    bk = _sc["bk"]
                    bkv = bk[0:1, :].rearrange("p (b k) -> p b k", b=NB)
                    nc.vector.tensor_tensor(out=bkv, in0=Ev[:, :, 10:20], in1=k2v,
                                            op=ALU.mult)
                    A_ = _sc["A_"]
                    Av = A_[0:1, :].rearrange("p (b k) -> p b k", b=NB)
                    bk2 = _sc["bk2"]
                    bk2v = bk2[0:1, :].rearrange("p (b k) -> p b k", b=NB)
                    nc.vector.tensor_tensor(out=bk2v, in0=bkv, in1=k2v, op=ALU.mult)
                    nc.vector.tensor_tensor(out=Av, in0=av[:, :, 0:10], in1=bk2v,
                                            op=ALU.subtract)
                    B_ = _sc["B_"]
                    nc.vector.tensor_scalar(out=B_[:], in0=bk[:], scalar1=2.0,
                                            scalar2=None, op0=ALU.mult)
                    C_ = _sc["C_"]
                    Cv = C_[0:1, :].rearrange("p (b k) -> p b k", b=NB)
                    nc.vector.tensor_scalar(out=Cv, in0=Ev[:, :, 10:20], scalar1=-1.0,
                                            scalar2=None, op0=ALU.mult)
                    # arg[u,(b,k)] = A + u*B + u^2*C  via 3 accumulating K=1 matmuls
                    nc.tensor.matmul(argp[:], sV3[0:1, 0:UC], A_[:],
                                     start=True, stop=False)
                    nc.tensor.matmul(argp[:], sV3[0:1, UC:2 * UC], B_[:],
                                     start=False, stop=False)
                    nc.tensor.matmul(argp[:], sV3[0:1, 2 * UC:3 * UC], C_[:],
                                     start=False, stop=True)
                    P = _sc["P"]
                    nc.scalar.activation(out=P[:], in_=argp[:], func=AF.Exp)
                    phi = _sc["phi"]
                    Pv = P[:, :].rearrange("p (b k) -> p b k", b=NB)
                    nc.vector.tensor_reduce(out=phi[:], in_=Pv,
                                            axis=mybir.AxisListType.X, op=ALU.add)
                    for b in range(NB):
                        nc.tensor.matmul(
                            wp[:, b : b + 1],
                            sOHB[:, NCHARS * b : NCHARS * b + NCHARS],
                            phi[:, b : b + 1],
                            start=True, stop=True,
                        )
                    # w -> xw tiles; xw1/xw2 first so Z2-rest's deps clear
                    # before Z1(t+1)'s (scheduler runs Z2-rest -> cell1 sooner)
                    for l in (1, 2, 0):
                        nc.vector.tensor_copy(out=xw[l][0:73, :], in_=wp[:])

                # ---- prologue: z1(0) ----
                # establish act table 0 before the loop so the in-loop
                # table-load analysis hoists (tanh+exp share one table)
                nc.scalar.activation(out=_sc["ti"][:], in_=sPB[:, 0:16],
                                     func=AF.Tanh)
                nc.vector.tensor_copy(out=xw[0][96:99, :], in_=sXT[0:3, 0:NB])
                layer_mms(0, sW1, mv_self(h[0]) + [(xw[0][:], 4)])

                with tc.For_i(0, T) as t:
                    cell(0)
                    # x_{t+1} -> xw0 (for next step's L1)
                    nc.vector.tensor_copy(out=xw[0][96:99, :],
                                          in_=sXT[0:3, bass.DynSlice(t * NB + NB, NB)])
                    # x_t -> xw1/xw2 (only needed by the L2/L3 rest passes)
                    nc.vector.tensor_copy(out=xw[1][96:99, :],
                                          in_=sXT[0:3, bass.ts(t, NB)])
                    nc.vector.tensor_copy(out=xw[2][96:99, :],
                                          in_=sXT[0:3, bass.ts(t, NB)])
                    # Z2/Z3 self-passes overlap cell0 / attention V-chain
                    layer_mms(1, sW2, mv_self(h[1]), cb=0)
                    attention_mms()
                    layer_mms(2, sW3, mv_self(h[2]), cb=0)
                    attention_tail(t)
                    # stage self halves + i/f peephole terms, off the
                    # attention chain (overlaps Z2-rest on the PE)
                    for _l in (1, 2):
                        pA = sPB[:, (3 * _l + 0) * 16 : (3 * _l + 0) * 16 + 16]
                        pB = sPB[:, (3 * _l + 1) * 16 : (3 * _l + 1) * 16 + 16]
                        nc.vector.tensor_tensor(out=PC[_l][:, 0:16], in0=pA,
                                                in1=c[_l][:], op=ALU.mult)
                        nc.vector.tensor_tensor(out=PC[_l][:, 16:32], in0=pB,
                                                in1=c[_l][:], op=ALU.mult)
                        nc.vector.tensor_tensor(out=zS[_l][:], in0=PC[_l][:],
                                                in1=z[_l][:, 0:64], op=ALU.add)
                    layer_mms(1, sW2, mv_skip(h[0], 4) + [(xw[1][:], 8)], cb=64)
                    cell(1, split=True)
                    # next step's L1 overlaps cell1 on the PE
                    layer_mms(0, sW1, mv_self(h[0]) + [(xw[0][:], 4)])
                    layer_mms(2, sW3, mv_skip(h[1], 4) + [(xw[2][:], 8)], cb=64)
                    # cell2 at body end: overlaps Z2-self/attz of step t+1
                    cell(2, t * 16 + 16, split=True)

            # ---- MDN head ----
            # Y1 rows: mu @0:40, eos @64, rho @96:116 ; Y2 rows: pi @0:20, s @32:72
            with tc.tile_pool(name="mpsum", bufs=2, space="PSUM") as mpsum, \
                 tc.tile_pool(name="mscr", bufs=2) as mscr, \
                 tc.tile_pool(name="mones", bufs=1) as mones:
                ones20 = mones.tile([NMIX, 1], F32)
                nc.vector.memset(ones20[:], 1.0)
                ones1_20 = mones.tile([1, NMIX], F32)
                nc.vector.memset(ones1_20[:], 1.0)
                # slot s = h3 of step s-1: steps 0..T-1 live in slots 1..T
                h3v = h3all[:, :].rearrange("p (t x) -> p t x", t=T + 1)
                CC = min(400, T * NB)
                TC = CC // NB
                for ch in range((T + TC - 1) // TC):
                    t0 = TC * ch
                    tn = min(TC, T - t0)
                    cc = tn * NB
                    yp1 = mpsum.tile([128, CC], F32, tag="yp1")
                    yp2 = mpsum.tile([72, CC], F32, tag="yp2")
                    for k in range(4):
                        nc.tensor.matmul(
                            yp1[0:128, 0:cc],
                            sWMDN[:, 200 * k : 200 * k + 128],
                            h3v[:, t0 + 1 : t0 + 1 + tn, 4 * k : 4 * k + 4],
                            start=(k == 0), stop=(k == 3))
                    for k in range(4):
                        nc.tensor.matmul(
                            yp2[0:72, 0:cc],
                            sWMDN[:, 200 * k + 128 : 200 * k + 200],
                            h3v[:, t0 + 1 : t0 + 1 + tn, 4 * k : 4 * k + 4],
                            start=(k == 0), stop=(k == 3))
                    o1 = OUTS1[:, NB * t0 : NB * t0 + cc]
                    o2 = OUTS2[:, NB * t0 : NB * t0 + cc]
                    # pi softmax (pi lives at yp2[0:20])
                    epi = mscr.tile([NMIX, CC], F32, tag="epi")
                    nc.scalar.activation(out=epi[0:NMIX, 0:cc], in_=yp2[0:NMIX, 0:cc],
                                         func=AF.Exp)
                    sp = mpsum.tile([1, CC], F32, tag="sp")
                    nc.tensor.matmul(sp[0:1, 0:cc], ones20[:], epi[0:NMIX, 0:cc],
                                     start=True, stop=True)
                    rec = mscr.tile([1, CC], F32, tag="rec")
                    nc.vector.reciprocal(out=rec[0:1, 0:cc], in_=sp[0:1, 0:cc])
                    bp = mpsum.tile([NMIX, CC], F32, tag="bp")
                    nc.tensor.matmul(bp[0:NMIX, 0:cc], ones1_20[:], rec[0:1, 0:cc],
                                     start=True, stop=True)
                    nc.vector.tensor_tensor(out=o2[0:20, :], in0=epi[0:NMIX, 0:cc],
                                            in1=bp[0:NMIX, 0:cc], op=ALU.mult)
                    # mu copy (yp1[0:40])
                    nc.vector.tensor_copy(out=o1[0:40, :], in_=yp1[0:40, 0:cc])
                    # rho tanh (yp1[96:116])
                    nc.scalar.activation(out=o1[96:116, :], in_=yp1[96:116, 0:cc],
                                         func=AF.Tanh)
                    # eos sigmoid via tanh (yp1[64:65])
                    teos = mscr.tile([65, CC], F32, tag="teos")
                    nc.scalar.activation(out=teos[64:65, 0:cc], in_=yp1[64:65, 0:cc],
                                         func=AF.Tanh, scale=0.5)
                    nc.vector.tensor_scalar(out=o1[64:65, :], in0=teos[64:65, 0:cc],
                                            scalar1=0.5, scalar2=0.5,
                                            op0=ALU.mult, op1=ALU.add)
                    # s exp (yp2[32:72], split at quadrant boundary)
                    nc.scalar.activation(out=o2[32:64, :], in_=yp2[32:64, 0:cc],
                                         func=AF.Exp)
                    nc.scalar.activation(out=o2[64:72, :], in_=yp2[64:72, 0:cc],
                                         func=AF.Exp)
            nc.gpsimd.dma_start(out=dOUT1[:], in_=OUTS1[:])
            nc.gpsimd.dma_start(out=dOUT2[:], in_=OUTS2[:])

    nc.compile()
    return nc


def _prep_core(inputs, bsl, T):
    x = np.asarray(inputs['input_strokes'], np.float32)
    chars = np.asarray(inputs['input_chars'])
    lens = np.asarray(inputs['input_char_lens'])

    def W_of(l):
        Wx = np.asarray(inputs['Wx%d' % l], np.float32)
        Wh = np.asarray(inputs['Wh%d' % l], np.float32)
        b = np.asarray(inputs['b%d' % l], np.float32)
        tiles = []
        # self-h blocks; blk3 128 rows: units at 0:16, bias at row 96
        for blk in range(3):
            tiles.append(Wh[128 * blk : 128 * blk + 128])
        t3 = np.zeros((128, 1600), np.float32)
        t3[0:16] = Wh[384:400]
        t3[96] = b
        tiles.append(t3)
        if l > 0:
            Wskip = Wx[76:476]
            for blk in range(3):
                tiles.append(Wskip[128 * blk : 128 * blk + 128])
            t7 = np.zeros((128, 1600), np.float32)
            t7[0:16] = Wskip[384:400]
            tiles.append(t7)
        # xw tile: w @0:73, x @96:99
        txw = np.zeros((XWROWS, 1600), np.float32)
        txw[0:73] = Wx[3:76]
        txw[96:99] = Wx[0:3]
        tiles.append(txw)
        return _pack_wblocks(tiles).astype(np.float16)

    W1, W2, W3 = W_of(0), W_of(1), W_of(2)

    PB = np.zeros((128, 9 * 16), np.float32)
    for l in range(3):
        p = np.asarray(inputs['p%d' % l], np.float32)
        for j in range(3):
            pbv = np.zeros((128, 16), np.float32)
            for blk in range(4):
                n = min(128, 400 - 128 * blk)
                pbv[0:n, 4 * blk : 4 * blk + 4] = p[j][128 * blk : 128 * blk + n, None]
            PB[:, (3 * l + j) * 16 : (3 * l + j) * 16 + 16] = pbv

    XT = np.zeros((3, (T + 1) * NB), np.float32)
    xs = x[bsl]
    for b in range(NB):
        XT[:, b:T * NB:NB] = xs[b].T
    # attention weights fp32; k-tile rows all 128, bias at row 96
    WATT = np.zeros((128, 4 * 30), np.float32)
    wa = np.asarray(inputs['W_att'], np.float32)
    for k in range(3):
        WATT[0:128, 30 * k : 30 * k + 30] = wa[128 * k : 128 * k + 128]
    WATT[0:16, 90:120] = wa[384:400]
    WATT[96, 90:120] = np.asarray(inputs['b_att'], np.float32)
    V3 = np.concatenate([np.ones(UC), np.arange(UC),
                         np.arange(UC) ** 2]).astype(np.float32)[None, :]
    OHB = np.zeros((UC, NB * NCHARS), np.float32)
    for b, gb in enumerate(bsl):
        oh = np.zeros((UC, NCHARS), np.float32)
        oh[np.arange(UC), chars[gb].astype(int)] = 1.0
        oh[int(lens[gb]):] = 0.0
        OHB[:, NCHARS * b : NCHARS * b + NCHARS] = oh
    # WMDN: per k-tile block [m1(128) | m2(72)]
    wm = np.asarray(inputs['W_mdn'], np.float32)
    bm = np.asarray(inputs['b_mdn'], np.float32)
    wmf = np.zeros((512, 121), np.float32)
    wmf[0:400] = wm
    wmf[3 * 128 + 96] = bm                  # bias via h3all p96 blk3 == 1.0
    m1 = np.zeros((512, 128), np.float32)
    m2 = np.zeros((512, 72), np.float32)
    m1[:, 0:40] = wmf[:, 20:60]             # mu1, mu2
    m1[:, 64:65] = wmf[:, 120:121]          # eos
    m1[:, 96:116] = wmf[:, 100:120]         # rho
    m2[:, 0:20] = wmf[:, 0:20]              # pi
    m2[:, 32:72] = wmf[:, 60:100]           # s1, s2
    WMDN = np.zeros((128, 4 * 200), np.float32)
    for k in range(4):
        WMDN[:, 200 * k : 200 * k + 128] = m1[128 * k : 128 * k + 128]
        WMDN[:, 200 * k + 128 : 200 * k + 200] = m2[128 * k : 128 * k + 128]
    HB = np.zeros((128, 16), np.float32)
    HB[96, 12:16] = 1.0
    HBR = np.zeros((128, 512), np.float32)
    for b in range(NB):
        HBR[96, 384 + 32 * b] = 1.0         # h1r bias for att blk3 k-tile
    return {'W1': W1, 'W2': W2, 'W3': W3, 'PB': PB, 'XT': XT, 'WATT': WATT,
            'V3': V3, 'OHB': OHB, 'WMDN': WMDN,
            'HB': HB.astype(np.float16), 'HBR': HBR}


def kernel(**inputs):
    x = np.asarray(inputs['input_strokes'])
    B, T, _ = x.shape
    if T not in _CACHE:
        _CACHE[T] = _build_program(T)
    nc = _CACHE[T]
    in_maps = [_prep_core(inputs, list(range(cr * NB, cr * NB + NB)), T)
               for cr in range(NCORES)]
    res = run_bass_kernel_spmd(nc, in_maps, list(range(NCORES)))
    outs = []
    for cr in range(NCORES):
        O1 = res.results[cr]['OUT1'].reshape(128, T, NB)
        O2 = res.results[cr]['OUT2'].reshape(72, T, NB)
        y = np.empty((NB, T, NOUT), np.float32)
        y[..., 0:20] = O2[0:20].transpose(2, 1, 0)
        y[..., 20:60] = O1[0:40].transpose(2, 1, 0)
        y[..., 60:100] = O2[32:72].transpose(2, 1, 0)
        y[..., 100:120] = O1[96:116].transpose(2, 1, 0)
        y[..., 120:121] = O1[64:65].transpose(2, 1, 0)
        outs.append(y)
    return np.concatenate(outs, 0).astype(np.float32)


# revision 25
# speedup vs baseline: 1.1903x; 1.1903x over previous
import sys
sys.path.insert(0, '/opt/trn_rl_repo')
import numpy as np
import concourse.bass as bass
import concourse.bacc as bacc
import concourse.tile as tile
from concourse import mybir
from concourse.bass_utils import run_bass_kernel_spmd

F32 = mybir.dt.float32
F16 = mybir.dt.float16
AF = mybir.ActivationFunctionType
ALU = mybir.AluOpType

U = 400       # LSTM units
KATT = 10     # attention gaussians
NCHARS = 73   # alphabet
NMIX = 20     # GMM components
UC = 50       # char positions
NB = 4        # batch per core
NCORES = 8
NOUT = 6 * NMIX + 1  # 121

# m-tiles: per gate [128,128,128,16] -> 16 m-tiles, psum z [128, 64]
MW = [128, 128, 128, 16] * 4
SOFF = [400 * (m // 4) + 128 * (m % 4) for m in range(16)]   # W col offsets (unpadded)
XWROWS = 99  # xw moving tile: w @0:73, junk, x @96:99 (x dst partition must be %32)

_CACHE = {}


def _pack_wblocks(tiles):
    """list of [rows<=128, 1600] -> [128, ntiles*1600] k-major."""
    out = np.zeros((128, len(tiles) * 1600), np.float32)
    for k, tl in enumerate(tiles):
        out[0 : tl.shape[0], k * 1600 : k * 1600 + 1600] = tl
    return out


def _build_program(T):
    nc = bacc.Bacc("TRN2", target_bir_lowering=False, debug=False, num_devices=NCORES)

    dW1 = nc.dram_tensor("W1", [128, 5 * 1600], F16, kind="ExternalInput").ap()
    dW2 = nc.dram_tensor("W2", [128, 9 * 1600], F16, kind="ExternalInput").ap()
    dW3 = nc.dram_tensor("W3", [128, 9 * 1600], F16, kind="ExternalInput").ap()
    dPB = nc.dram_tensor("PB", [128, 9 * 16], F32, kind="ExternalInput").ap()
    dXT = nc.dram_tensor("XT", [3, (T + 1) * NB], F32, kind="ExternalInput").ap()
    dWATT = nc.dram_tensor("WATT", [128, 4 * 30], F32, kind="ExternalInput").ap()
    dV3 = nc.dram_tensor("V3", [1, 3 * UC], F32, kind="ExternalInput").ap()
    dOHB = nc.dram_tensor("OHB", [UC, NB * NCHARS], F32, kind="ExternalInput").ap()
    dWMDN = nc.dram_tensor("WMDN", [128, 4 * 200], F32, kind="ExternalInput").ap()
    dHB = nc.dram_tensor("HB", [128, 16], F16, kind="ExternalInput").ap()
    dHBR = nc.dram_tensor("HBR", [128, 512], F32, kind="ExternalInput").ap()
    dOUT1 = nc.dram_tensor("OUT1", [128, T * NB], F32, kind="ExternalOutput").ap()
    dOUT2 = nc.dram_tensor("OUT2", [72, T * NB], F32, kind="ExternalOutput").ap()

    with tile.TileContext(nc) as tc:
        with tc.tile_pool(name="statics", bufs=1) as statics, \
             tc.tile_pool(name="states", bufs=1) as states:

            sW1 = statics.tile([128, 5 * 1600], F16)
            sW2 = statics.tile([128, 9 * 1600], F16)
            sW3 = statics.tile([128, 9 * 1600], F16)
            sPB = statics.tile([128, 9 * 16], F32)
            sXT = statics.tile([3, (T + 1) * NB], F32)
            sWATT = statics.tile([128, 4 * 30], F32)
            sV3 = statics.tile([1, 3 * UC], F32)
            sOHB = statics.tile([UC, NB * NCHARS], F32)
            sWMDN = statics.tile([128, 4 * 200], F32)
            sHB = statics.tile([128, 16], F16)

            # h3all has T+1 slots: slot s holds h3 of step s-1 (cell2 deferred)
            h3all = states.tile([128, (T + 1) * 16], F32)
            OUTS1 = states.tile([128, T * NB], F32)
            OUTS2 = states.tile([72, T * NB], F32)
            h = [states.tile([128, 16], F16, name=f"h{i}") for i in range(3)]
            # h1 replica, batch at col {128k + 32b}: att matmul stationary
            h1r = states.tile([128, 512], F32, name="h1r")
            c = [states.tile([128, 16], F32, name=f"c{i}") for i in range(3)]
            xw = [states.tile([XWROWS, NB], F16, name=f"xw{i}") for i in range(3)]
            kap = states.tile([1, 4 * KATT], F32)  # (b, k)

            for dst, src in [(sW1, dW1), (sW2, dW2), (sW3, dW3), (sPB, dPB),
                             (sXT, dXT), (sWATT, dWATT), (sV3, dV3),
                             (sOHB, dOHB), (sWMDN, dWMDN), (sHB, dHB),
                             (h1r, dHBR)]:
                nc.gpsimd.dma_start(out=dst[:], in_=src[:])

            for tl in c + xw:
                nc.vector.memset(tl[:], 0.0)
            nc.vector.memset(kap[:], 0.0)
            nc.vector.memset(h3all[:], 0.0)
            # h init: zeros except bias constant 1.0 at (p96, blk3 cols)
            for tl in h:
                nc.vector.tensor_copy(out=tl[:], in_=sHB[:])
            nc.vector.memset(h3all[96:128, :], 1.0)

            with tc.tile_pool(name="psum", bufs=1, space="PSUM") as psum, \
                 tc.tile_pool(name="scratch", bufs=2) as scratch:

                # full-bank z tiles (512 f32 = 2KB) so each owns its zero region
                z = [psum.tile([128, 512], F32, name=f"z{i}") for i in range(3)]
                attp = psum.tile([97, 30], F32)   # att out, batch b at row 32b
                argp = psum.tile([UC, NB * KATT], F32)
                wp = psum.tile([NCHARS, NB], F32)
                for tl in z:
                    nc.vector.memset(tl[:], 0.0)

                zS = {1: None, 2: None}
                _sc = {}
                for _t in ["pc0", "zi_", "ti", "ig", "pc1", "zf_", "tf", "fg",
                           "tg", "m1", "m2", "pc2", "zo_", "to", "og", "tcn",
                           "zi2", "zf2", "zg2", "zo2", "zgc"]:
                    _sc[_t] = scratch.tile([128, 16], F32, tag=_t, name=_t)
                PC = {1: None, 2: None}
                for _i in (1, 2):
                    zS[_i] = scratch.tile([128, 64], F32, tag=f"zS{_i}",
                                          name=f"zS{_i}")
                    nc.vector.memset(zS[_i][:], 0.0)
                    PC[_i] = scratch.tile([128, 64], F32, tag=f"PC{_i}",
                                          name=f"PC{_i}")
                    nc.vector.memset(PC[_i][:], 0.0)
                for _t, _shp in [("attR", [1, NB * 30]), ("E", [1, NB * 30]),
                                 ("kap2", [1, NB * KATT]),
                                 ("bk", [1, NB * KATT]), ("A_", [1, NB * KATT]),
                                 ("bk2", [1, NB * KATT]), ("B_", [1, NB * KATT]),
                                 ("C_", [1, NB * KATT]), ("P", [UC, NB * KATT]),
                                 ("phi", [UC, NB])]:
                    _sc[_t] = scratch.tile(_shp, F32, tag=_t, name=_t)

                def st(tag):
                    return _sc[tag]

                def layer_mms(l, sW, movs, cb=0):
                    # per-m-tile accumulation groups (start/stop per m): the
                    # group check serializes groups, preserving long
                    # same-psum-address runs (HW chains those at ~27ns;
                    # address switches cost ~95ns pipeline drain). cb: psum
                    # column base — L2/L3 self and rest passes write separate
                    # halves of the z bank, summed by the (split) cell.
                    for m in range(16):
                        w0, mw = int(SOFF[m]), MW[m]
                        for ki, (mov, kidx) in enumerate(movs):
                            kp = mov.shape[0]
                            nc.tensor.matmul(
                                z[l][0:mw, cb + 4 * m : cb + 4 * m + 4],
                                sW[0:kp, kidx * 1600 + w0 : kidx * 1600 + w0 + mw],
                                mov,
                                start=(ki == 0),
                                stop=(ki == len(movs) - 1),
                            )

                def mv_self(hh):
                    return [(hh[:, 0:4], 0), (hh[:, 4:8], 1),
                            (hh[:, 8:12], 2), (hh[:, 12:16], 3)]

                def mv_skip(hh, base):
                    return [(hh[:, 0:4], base), (hh[:, 4:8], base + 1),
                            (hh[:, 8:12], base + 2), (hh[:, 12:16], base + 3)]

                def cell(l, slot16=None, split=False):
                    zt = z[l]
                    zRi, zRf = zt[:, 64:80], zt[:, 80:96]
                    zRg, zRo = zt[:, 96:112], zt[:, 112:128]
                    p2 = sPB[:, (3 * l + 2) * 16 : (3 * l + 2) * 16 + 16]
                    ct = c[l]
                    if split:
                        # zS staged to SBUF earlier (during attention): one
                        # on-chain add per gate, one PSUM operand each
                        zs = zS[l]
                        zi_ = st("zi_")
                        nc.vector.tensor_tensor(out=zi_[:], in0=zs[:, 0:16],
                                                in1=zRi, op=ALU.add)
                        zf_ = st("zf_")
                        nc.vector.tensor_tensor(out=zf_[:], in0=zs[:, 16:32],
                                                in1=zRf, op=ALU.add)
                        zg_ = st("zg2")
                        nc.vector.tensor_tensor(out=zg_[:], in0=zs[:, 32:48],
                                                in1=zRg, op=ALU.add)
                        # o-gate partial (peephole p2*c_new added later)
                        zo12 = st("zo2")
                        nc.vector.tensor_tensor(out=zo12[:], in0=zs[:, 48:64],
                                                in1=zRo, op=ALU.add)
                        zi_, zf_, zg_, zo_base = zi_[:], zf_[:], zg_[:], zo12[:]
                    else:
                        p0 = sPB[:, (3 * l + 0) * 16 : (3 * l + 0) * 16 + 16]
                        p1 = sPB[:, (3 * l + 1) * 16 : (3 * l + 1) * 16 + 16]
                        pc0 = st("pc0")
                        nc.vector.tensor_tensor(out=pc0[:], in0=p0, in1=ct[:],
                                                op=ALU.mult)
                        pc1 = st("pc1")
                        nc.vector.tensor_tensor(out=pc1[:], in0=p1, in1=ct[:],
                                                op=ALU.mult)
                        zi_ = st("zi_")
                        nc.vector.tensor_tensor(out=zi_[:], in0=pc0[:],
                                                in1=zt[:, 0:16], op=ALU.add)
                        zf_ = st("zf_")
                        nc.vector.tensor_tensor(out=zf_[:], in0=pc1[:],
                                                in1=zt[:, 16:32], op=ALU.add)
                        zi_, zf_ = zi_[:], zf_[:]
                        zg_, zo_base = zt[:, 32:48], zt[:, 48:64]
                    ti = st("ti")
                    nc.scalar.activation(out=ti[:], in_=zi_, func=AF.Tanh, scale=0.5)
                    tf = st("tf")
                    nc.scalar.activation(out=tf[:], in_=zf_, func=AF.Tanh, scale=0.5)
                    tg = st("tg")
                    nc.scalar.activation(out=tg[:], in_=zg_, func=AF.Tanh)
                    ig = st("ig")
                    nc.vector.tensor_scalar(out=ig[:], in0=ti[:], scalar1=0.5,
                                            scalar2=0.5, op0=ALU.mult, op1=ALU.add)
                    fg = st("fg")
                    nc.vector.tensor_scalar(out=fg[:], in0=tf[:], scalar1=0.5,
                                            scalar2=0.5, op0=ALU.mult, op1=ALU.add)
                    m1 = st("m1")
                    nc.vector.tensor_tensor(out=m1[:], in0=ig[:], in1=tg[:], op=ALU.mult)
                    m2 = st("m2")
                    nc.vector.tensor_tensor(out=m2[:], in0=fg[:], in1=ct[:], op=ALU.mult)
                    nc.vector.tensor_tensor(out=ct[:], in0=m1[:], in1=m2[:], op=ALU.add)
                    tcn = st("tcn")
                    nc.scalar.activation(out=tcn[:], in_=ct[:], func=AF.Tanh)
                    pc2 = st("pc2")
                    nc.vector.tensor_tensor(out=pc2[:], in0=p2, in1=ct[:], op=ALU.mult)
                    zo_ = st("zo_")
                    nc.vector.tensor_tensor(out=zo_[:], in0=pc2[:], in1=zo_base,
                                            op=ALU.add)
                    to = st("to")
                    nc.scalar.activation(out=to[:], in_=zo_[:], func=AF.Tanh, scale=0.5)
                    og = st("og")
                    nc.vector.tensor_scalar(out=og[:], in0=to[:], scalar1=0.5,
                                            scalar2=0.5, op0=ALU.mult, op1=ALU.add)
                    if l == 0:
                        # fp32 replica first: it gates the attention matmuls
                        hrb = h1r[:, 0:384].rearrange(
                            "p (k b s) -> p k b s", k=3, b=4)[:, :, :, 0:1]
                        ogb = og[:, 0:12].rearrange(
                            "p (k b s) -> p k b s", k=3, b=4)
                        tcb = tcn[:, 0:12].rearrange(
                            "p (k b s) -> p k b s", k=3, b=4)
                        nc.vector.tensor_tensor(out=hrb, in0=ogb, in1=tcb,
                                                op=ALU.mult)
                        hrr = h1r[0:16, 384:512].rearrange(
                            "p (b s) -> p b s", b=4)[:, :, 0:1]
                        ogr = og[0:16, 12:16].rearrange("p (b s) -> p b s", b=4)
                        tcr = tcn[0:16, 12:16].rearrange("p (b s) -> p b s", b=4)
                        nc.vector.tensor_tensor(out=hrr, in0=ogr, in1=tcr,
                                                op=ALU.mult)
                    # h = og*tcn; blk3 valid at partitions 0:16 (bias at p96)
                    nc.vector.tensor_tensor(out=h[l][:, 0:12], in0=og[:, 0:12],
                                            in1=tcn[:, 0:12], op=ALU.mult)
                    nc.vector.tensor_tensor(out=h[l][0:16, 12:16], in0=og[0:16, 12:16],
                                            in1=tcn[0:16, 12:16], op=ALU.mult)
                    if l == 2:
                        # fp32 h3 straight into its h3all slot
                        nc.vector.tensor_tensor(
                            out=h3all[:, bass.DynSlice(slot16, 12)],
                            in0=og[:, 0:12], in1=tcn[:, 0:12], op=ALU.mult)
                        nc.vector.tensor_tensor(
                            out=h3all[0:16, bass.DynSlice(slot16 + 12, 4)],
                            in0=og[0:16, 12:16], in1=tcn[0:16, 12:16], op=ALU.mult)

                def attention_mms():
                    # attp[32b + junk, 30] = h1r_k^T @ WATT_k (fp32, 4 fat mms)
                    for k in range(4):
                        stat = (h1r[0:128, 128 * k : 128 * k + 97] if k < 3
                                else h1r[0:128, 384:481])
                        kp = stat.shape[0]
                        nc.tensor.matmul(
                            attp[0:97, 0:30],
                            stat,
                            sWATT[0:kp, 30 * k : 30 * k + 30],
                            start=(k == 0), stop=(k == 3),
                        )

                def attention_tail(t):
                    attR = _sc["attR"]
                    for b in range(NB):
                        nc.vector.tensor_copy(
                            out=attR[0:1, 30 * b : 30 * b + 30],
                            in_=attp[32 * b : 32 * b + 1, 0:30])
                    E = _sc["E"]
                    nc.scalar.activation(out=E[:], in_=attR[:], func=AF.Exp)
                    Ev = E[0:1, :].rearrange("p (b x) -> p b x", b=NB)
                    av = attR[0:1, :].rearrange("p (b x) -> p b x", b=NB)
                    kapv = kap[0:1, :].rearrange("p (b k) -> p b k", b=NB)
                    kap2 = _sc["kap2"]
                    k2v = kap2[0:1, :].rearrange("p (b k) -> p b k", b=NB)
                    nc.vector.tensor_tensor(out=k2v, in0=kapv, in1=Ev[:, :, 20:30],
                                            op=ALU.add)
                    nc.vector.tensor_copy(out=kap[:], in_=kap2[:])
                    bk = _sc["bk"]
                    bkv = bk[0:1, :].rearrange("p (b k) -> p b k", b=NB)
                    nc.vector.tensor_tensor(out=bkv, in0=Ev[:, :, 10:20], in1=k2v,
                                            op=ALU.mult)
                    A_ = _sc["A_"]
                    Av = A_[0:1, :].rearrange("p (b k) -> p b k", b=NB)
                    bk2 = _sc["bk2"]
                    bk2v = bk2[0:1, :].rearrange("p (b k) -> p b k", b=NB)
                    nc.vector.tensor_tensor(out=bk2v, in0=bkv, in1=k2v, op=ALU.mult)
                    nc.vector.tensor_tensor(out=Av, in0=av[:, :, 0:10], in1=bk2v,
                                            op=ALU.subtract)
                    B_ = _sc["B_"]
                    nc.vector.tensor_scalar(out=B_[:], in0=bk[:], scalar1=2.0,
                                            scalar2=None, op0=ALU.mult)
                    C_ = _sc["C_"]
                    Cv = C_[0:1, :].rearrange("p (b k) -> p b k", b=NB)
                    nc.vector.tensor_scalar(out=Cv, in0=Ev[:, :, 10:20], scalar1=-1.0,
                                            scalar2=None, op0=ALU.mult)
                    # arg[u,(b,k)] = A + u*B + u^2*C  via 3 accumulating K=1 matmuls
                    nc.tensor.matmul(argp[:], sV3[0:1, 0:UC], A_[:],
                                     start=True, stop=False)
                    nc.tensor.matmul(argp[:], sV3[0:1, UC:2 * UC], B_[:],
                                     start=False, stop=False)
                    nc.tensor.matmul(argp[:], sV3[0:1, 2 * UC:3 * UC], C_[:],
                                     start=False, stop=True)
                    P = _sc["P"]
                    nc.scalar.activation(out=P[:], in_=argp[:], func=AF.Exp)
                    phi = _sc["phi"]
                    Pv = P[:, :].rearrange("p (b k) -> p b k", b=NB)
                    nc.vector.tensor_reduce(out=phi[:], in_=Pv,
                                            axis=mybir.AxisListType.X, op=ALU.add)
                    for b in range(NB):
                        nc.tensor.matmul(
                            wp[:, b : b + 1],
                            sOHB[:, NCHARS * b : NCHARS * b + NCHARS],
                            phi[:, b : b + 1],
                            start=True, stop=True,
                        )
                    # w -> xw tiles; xw1/xw2 first so Z2-rest's deps clear
                    # before Z1(t+1)'s (scheduler runs Z2-rest -> cell1 sooner)
                    for l in (1, 2, 0):
                        nc.vector.tensor_copy(out=xw[l][0:73, :], in_=wp[:])

                # ---- prologue: z1(0) ----
                # establish act table 0 before the loop so the in-loop
                # table-load analysis hoists (tanh+exp share one table)
                nc.scalar.activation(out=_sc["ti"][:], in_=sPB[:, 0:16],
                                     func=AF.Tanh)
                nc.vector.tensor_copy(out=xw[0][96:99, :], in_=sXT[0:3, 0:NB])
                layer_mms(0, sW1, mv_self(h[0]) + [(xw[0][:], 4)])

                with tc.For_i(0, T) as t:
                    cell(0)
                    # deferred cell2 of step t-1 (slot t); at t=0 it is a
                    # zero-preserving dummy (z3 == 0 -> h3,c3 stay 0)
                    cell(2, t * 16, split=True)
                    # x_{t+1} -> xw0 (for next step's L1)
                    nc.vector.tensor_copy(out=xw[0][96:99, :],
                                          in_=sXT[0:3, bass.DynSlice(t * NB + NB, NB)])
                    # x_t -> xw1/xw2 (only needed by the L2/L3 rest passes)
                    nc.vector.tensor_copy(out=xw[1][96:99, :],
                                          in_=sXT[0:3, bass.ts(t, NB)])
                    nc.vector.tensor_copy(out=xw[2][96:99, :],
                                          in_=sXT[0:3, bass.ts(t, NB)])
                    # Z2/Z3 self-passes overlap cell0 / attention V-chain
                    layer_mms(1, sW2, mv_self(h[1]), cb=0)
                    attention_mms()
                    layer_mms(2, sW3, mv_self(h[2]), cb=0)
                    attention_tail(t)
                    # stage self halves + i/f peephole terms, off the
                    # attention chain (overlaps Z2-rest on the PE)
                    for _l in (1, 2):
                        pA = sPB[:, (3 * _l + 0) * 16 : (3 * _l + 0) * 16 + 16]
                        pB = sPB[:, (3 * _l + 1) * 16 : (3 * _l + 1) * 16 + 16]
                        nc.vector.tensor_tensor(out=PC[_l][:, 0:16], in0=pA,
                                                in1=c[_l][:], op=ALU.mult)
                        nc.vector.tensor_tensor(out=PC[_l][:, 16:32], in0=pB,
                                                in1=c[_l][:], op=ALU.mult)
                        nc.vector.tensor_tensor(out=zS[_l][:], in0=PC[_l][:],
                                                in1=z[_l][:, 0:64], op=ALU.add)
                    layer_mms(1, sW2, mv_skip(h[0], 4) + [(xw[1][:], 8)], cb=64)
                    cell(1, split=True)
                    # next step's L1 overlaps cell1 on the PE
                    layer_mms(0, sW1, mv_self(h[0]) + [(xw[0][:], 4)])
                    layer_mms(2, sW3, mv_skip(h[1], 4) + [(xw[2][:], 8)], cb=64)

                # epilogue: last deferred cell2 (step T-1 -> slot T)
                cell(2, T * 16, split=True)

            # ---- MDN head ----
            # Y1 rows: mu @0:40, eos @64, rho @96:116 ; Y2 rows: pi @0:20, s @32:72
            with tc.tile_pool(name="mpsum", bufs=2, space="PSUM") as mpsum, \
                 tc.tile_pool(name="mscr", bufs=2) as mscr, \
                 tc.tile_pool(name="mones", bufs=1) as mones:
                ones20 = mones.tile([NMIX, 1], F32)
                nc.vector.memset(ones20[:], 1.0)
                ones1_20 = mones.tile([1, NMIX], F32)
                nc.vector.memset(ones1_20[:], 1.0)
                # slot s = h3 of step s-1: steps 0..T-1 live in slots 1..T
                h3v = h3all[:, :].rearrange("p (t x) -> p t x", t=T + 1)
                CC = min(400, T * NB)
                TC = CC // NB
                for ch in range((T + TC - 1) // TC):
                    t0 = TC * ch
                    tn = min(TC, T - t0)
                    cc = tn * NB
                    yp1 = mpsum.tile([128, CC], F32, tag="yp1")
                    yp2 = mpsum.tile([72, CC], F32, tag="yp2")
                    for k in range(4):
                        nc.tensor.matmul(
                            yp1[0:128, 0:cc],
                            sWMDN[:, 200 * k : 200 * k + 128],
                            h3v[:, t0 + 1 : t0 + 1 + tn, 4 * k : 4 * k + 4],
                            start=(k == 0), stop=(k == 3))
                    for k in range(4):
                        nc.tensor.matmul(
                            yp2[0:72, 0:cc],
                            sWMDN[:, 200 * k + 128 : 200 * k + 200],
                            h3v[:, t0 + 1 : t0 + 1 + tn, 4 * k : 4 * k + 4],
                            start=(k == 0), stop=(k == 3))
                    o1 = OUTS1[:, NB * t0 : NB * t0 + cc]
                    o2 = OUTS2[:, NB * t0 : NB * t0 + cc]
                    # pi softmax (pi lives at yp2[0:20])
                    epi = mscr.tile([NMIX, CC], F32, tag="epi")
                    nc.scalar.activation(out=epi[0:NMIX, 0:cc], in_=yp2[0:NMIX, 0:cc],
                                         func=AF.Exp)
                    sp = mpsum.tile([1, CC], F32, tag="sp")
                    nc.tensor.matmul(sp[0:1, 0:cc], ones20[:], epi[0:NMIX, 0:cc],
                                     start=True, stop=True)
                    rec = mscr.tile([1, CC], F32, tag="rec")
                    nc.vector.reciprocal(out=rec[0:1, 0:cc], in_=sp[0:1, 0:cc])
                    bp = mpsum.tile([NMIX, CC], F32, tag="bp")
                    nc.tensor.matmul(bp[0:NMIX, 0:cc], ones1_20[:], rec[0:1, 0:cc],
                                     start=True, stop=True)
                    nc.vector.tensor_tensor(out=o2[0:20, :], in0=epi[0:NMIX, 0:cc],
                                            in1=bp[0:NMIX, 0:cc], op=ALU.mult)
                    # mu copy (yp1[0:40])
                    nc.vector.tensor_copy(out=o1[0:40, :], in_=yp1[0:40, 0:cc])
                    # rho tanh (yp1[96:116])
                    nc.scalar.activation(out=o1[96:116, :], in_=yp1[96:116, 0:cc],
                                         func=AF.Tanh)
                    # eos sigmoid via tanh (yp1[64:65])
                    teos = mscr.tile([65, CC], F32, tag="teos")
                    nc.scalar.activation(out=teos[64:65, 0:cc], in_=yp1[64:65, 0:cc],
                                         func=AF.Tanh, scale=0.5)
                    nc.vector.tensor_scalar(out=o1[64:65, :], in0=teos[64:65, 0:cc],
                                            scalar1=0.5, scalar2=0.5,
                                            op0=ALU.mult, op1=ALU.add)
                    # s exp (yp2[32:72], split at quadrant boundary)
                    nc.scalar.activation(out=o2[32:64, :], in_=yp2[32:64, 0:cc],
                                         func=AF.Exp)
                    nc.scalar.activation(out=o2[64:72, :], in_=yp2[64:72, 0:cc],
                                         func=AF.Exp)
            nc.gpsimd.dma_start(out=dOUT1[:], in_=OUTS1[:])
            nc.gpsimd.dma_start(out=dOUT2[:], in_=OUTS2[:])

    nc.compile()
    return nc


def _prep_core(inputs, bsl, T):
    x = np.asarray(inputs['input_strokes'], np.float32)
    chars = np.asarray(inputs['input_chars'])
    lens = np.asarray(inputs['input_char_lens'])

    def W_of(l):
        Wx = np.asarray(inputs['Wx%d' % l], np.float32)
        Wh = np.asarray(inputs['Wh%d' % l], np.float32)
        b = np.asarray(inputs['b%d' % l], np.float32)
        tiles = []
        # self-h blocks; blk3 128 rows: units at 0:16, bias at row 96
        for blk in range(3):
            tiles.append(Wh[128 * blk : 128 * blk + 128])
        t3 = np.zeros((128, 1600), np.float32)
        t3[0:16] = Wh[384:400]
        t3[96] = b
        tiles.append(t3)
        if l > 0:
            Wskip = Wx[76:476]
            for blk in range(3):
                tiles.append(Wskip[128 * blk : 128 * blk + 128])
            t7 = np.zeros((128, 1600), np.float32)
            t7[0:16] = Wskip[384:400]
            tiles.append(t7)
        # xw tile: w @0:73, x @96:99
        txw = np.zeros((XWROWS, 1600), np.float32)
        txw[0:73] = Wx[3:76]
        txw[96:99] = Wx[0:3]
        tiles.append(txw)
        return _pack_wblocks(tiles).astype(np.float16)

    W1, W2, W3 = W_of(0), W_of(1), W_of(2)

    PB = np.zeros((128, 9 * 16), np.float32)
    for l in range(3):
        p = np.asarray(inputs['p%d' % l], np.float32)
        for j in range(3):
            pbv = np.zeros((128, 16), np.float32)
            for blk in range(4):
                n = min(128, 400 - 128 * blk)
                pbv[0:n, 4 * blk : 4 * blk + 4] = p[j][128 * blk : 128 * blk + n, None]
            PB[:, (3 * l + j) * 16 : (3 * l + j) * 16 + 16] = pbv

    XT = np.zeros((3, (T + 1) * NB), np.float32)
    xs = x[bsl]
    for b in range(NB):
        XT[:, b:T * NB:NB] = xs[b].T
    # attention weights fp32; k-tile rows all 128, bias at row 96
    WATT = np.zeros((128, 4 * 30), np.float32)
    wa = np.asarray(inputs['W_att'], np.float32)
    for k in range(3):
        WATT[0:128, 30 * k : 30 * k + 30] = wa[128 * k : 128 * k + 128]
    WATT[0:16, 90:120] = wa[384:400]
    WATT[96, 90:120] = np.asarray(inputs['b_att'], np.float32)
    V3 = np.concatenate([np.ones(UC), np.arange(UC),
                         np.arange(UC) ** 2]).astype(np.float32)[None, :]
    OHB = np.zeros((UC, NB * NCHARS), np.float32)
    for b, gb in enumerate(bsl):
        oh = np.zeros((UC, NCHARS), np.float32)
        oh[np.arange(UC), chars[gb].astype(int)] = 1.0
        oh[int(lens[gb]):] = 0.0
        OHB[:, NCHARS * b : NCHARS * b + NCHARS] = oh
    # WMDN: per k-tile block [m1(128) | m2(72)]
    wm = np.asarray(inputs['W_mdn'], np.float32)
    bm = np.asarray(inputs['b_mdn'], np.float32)
    wmf = np.zeros((512, 121), np.float32)
    wmf[0:400] = wm
    wmf[3 * 128 + 96] = bm                  # bias via h3all p96 blk3 == 1.0
    m1 = np.zeros((512, 128), np.float32)
    m2 = np.zeros((512, 72), np.float32)
    m1[:, 0:40] = wmf[:, 20:60]             # mu1, mu2
    m1[:, 64:65] = wmf[:, 120:121]          # eos
    m1[:, 96:116] = wmf[:, 100:120]         # rho
    m2[:, 0:20] = wmf[:, 0:20]              # pi
    m2[:, 32:72] = wmf[:, 60:100]           # s1, s2
    WMDN = np.zeros((128, 4 * 200), np.float32)
    for k in range(4):
        WMDN[:, 200 * k : 200 * k + 128] = m1[128 * k : 128 * k + 128]
        WMDN[:, 200 * k + 128 : 200 * k + 200] = m2[128 * k : 128 * k + 128]
    HB = np.zeros((128, 16), np.float32)
    HB[96, 12:16] = 1.0
    HBR = np.zeros((128, 512), np.float32)
    for b in range(NB):
        HBR[96, 384 + 32 * b] = 1.0         # h1r bias for att blk3 k-tile
    return {'W1': W1, 'W2': W2, 'W3': W3, 'PB': PB, 'XT': XT, 'WATT': WATT,
            'V3': V3, 'OHB': OHB, 'WMDN': WMDN,
            'HB': HB.astype(np.float16), 'HBR': HBR}


def kernel(**inputs):
    x = np.asarray(inputs['input_strokes'])
    B, T, _ = x.shape
    if T not in _CACHE:
        _CACHE[T] = _build_program(T)
    nc = _CACHE[T]
    in_maps = [_prep_core(inputs, list(range(cr * NB, cr * NB + NB)), T)
               for cr in range(NCORES)]
    res = run_bass_kernel_spmd(nc, in_maps, list(range(NCORES)))
    outs = []
    for cr in range(NCORES):
        O1 = res.results[cr]['OUT1'].reshape(128, T, NB)
        O2 = res.results[cr]['OUT2'].reshape(72, T, NB)
        y = np.empty((NB, T, NOUT), np.float32)
        y[..., 0:20] = O2[0:20].transpose(2, 1, 0)
        y[..., 20:60] = O1[0:40].transpose(2, 1, 0)
        y[..., 60:100] = O2[32:72].transpose(2, 1, 0)
        y[..., 100:120] = O1[96:116].transpose(2, 1, 0)
        y[..., 120:121] = O1[64:65].transpose(2, 1, 0)
        outs.append(y)
    return np.concatenate(outs, 0).astype(np.float32)
